# revision 5
# baseline (speedup 1.0000x reference)
"""CosineEmbeddingLoss-style kernel for Trainium2 (Bass/Tile), 8-core data parallel.

reference semantics (fp32):
    dot   = sum(x*y, -1); xx = sum(x*x, -1); yy = sum(y*y, -1)
    d     = dot / max(sqrt(xx*yy), EPS)
    per   = where(p == 1, 1 - d, max(0, d - MARGIN))
    loss  = sum(per)

Sharding: rows (N) split contiguously across 8 cores. Each core computes its
partial loss as a (1,1) f32; host sums the 8 partials.
"""

import numpy as np

import concourse.bacc as bacc
import concourse.tile as tile
from concourse import mybir
from concourse.bass_utils import run_bass_kernel_spmd

N, D = 32768, 1024
N_CORES = 8
ROWS_PER_CORE = N // N_CORES  # 4096
P = 128
CHUNK_ROWS = 512  # rows per dma_start (2 MiB per tensor per chunk)
MARGIN = 0.5
EPS = 1e-8

F32 = mybir.dt.float32
Alu = mybir.AluOpType
Act = mybir.ActivationFunctionType


def build(rows_per_core=ROWS_PER_CORE, d=D, chunk_rows=CHUNK_ROWS):
    n_chunks = rows_per_core // chunk_rows
    jtiles = chunk_rows // P          # row-tiles per chunk
    n_tiles = rows_per_core // P      # stats columns

    nc = bacc.Bacc(
        "TRN2",
        target_bir_lowering=False,
        debug=False,
        enable_asserts=False,
        num_devices=N_CORES,
    )
    x_dram = nc.dram_tensor("x", [rows_per_core, d], F32, kind="ExternalInput")
    y_dram = nc.dram_tensor("y", [rows_per_core, d], F32, kind="ExternalInput")
    m_dram = nc.dram_tensor("m", [P, n_tiles], mybir.dt.uint8, kind="ExternalInput")
    o_dram = nc.dram_tensor("out", [1, 1], F32, kind="ExternalOutput")

    with tile.TileContext(nc) as tc:
        with (
            tc.tile_pool(name="xin", bufs=3) as xpool,
            tc.tile_pool(name="yin", bufs=3) as ypool,
            tc.tile_pool(name="scratch", bufs=2) as spool,
            tc.tile_pool(name="stats", bufs=1) as statpool,
            tc.tile_pool(name="ep", bufs=1) as eppool,
            tc.tile_pool(name="psum", bufs=1, space="PSUM") as psumpool,
        ):
            dot_s = statpool.tile([P, n_tiles], F32)
            xx_s = statpool.tile([P, n_tiles], F32)
            yy_s = statpool.tile([P, n_tiles], F32)
            mask_t = statpool.tile([P, n_tiles], mybir.dt.uint8)
            ones_t = statpool.tile([P, 1], F32)
            zero_t = statpool.tile([P, 1], F32)
            negm_t = statpool.tile([P, 1], F32)
            nc.sync.dma_start(out=mask_t, in_=m_dram.ap())
            nc.vector.memset(ones_t, 1.0)
            nc.vector.memset(zero_t, 0.0)
            nc.vector.memset(negm_t, -MARGIN)

            xap = x_dram.ap()
            yap = y_dram.ap()
            for c in range(n_chunks):
                r0 = c * chunk_rows
                x_t = xpool.tile([P, jtiles, d], F32, tag="x")
                y_t = ypool.tile([P, jtiles, d], F32, tag="y")
                nc.sync.dma_start(
                    out=x_t,
                    in_=xap[r0 : r0 + chunk_rows, :].rearrange("(j p) d -> p j d", p=P),
                )
                nc.sync.dma_start(
                    out=y_t,
                    in_=yap[r0 : r0 + chunk_rows, :].rearrange("(j p) d -> p j d", p=P),
                )
                for j in range(jtiles):
                    t = c * jtiles + j
                    prod = spool.tile([P, d], F32, tag="prod")
                    nc.vector.scalar_tensor_tensor(
                        out=prod,
                        in0=x_t[:, j, :],
                        scalar=1.0,
                        in1=y_t[:, j, :],
                        op0=Alu.mult,
                        op1=Alu.mult,
                        accum_out=dot_s[:, t : t + 1],
                    )
                    junkx = spool.tile([P, d], F32, tag="junkx")
                    nc.scalar.activation(
                        out=junkx,
                        in_=x_t[:, j, :],
                        func=Act.Square,
                        bias=zero_t,
                        accum_out=xx_s[:, t : t + 1],
                    )
                    junky = spool.tile([P, d], F32, tag="junky")
                    nc.scalar.activation(
                        out=junky,
                        in_=y_t[:, j, :],
                        func=Act.Square,
                        bias=zero_t,
                        accum_out=yy_s[:, t : t + 1],
                    )

            # ---- epilogue on (P, n_tiles) stats ----
            pr = eppool.tile([P, n_tiles], F32)
            nc.vector.tensor_mul(pr, xx_s, yy_s)
            pr2 = eppool.tile([P, n_tiles], F32)
            nc.vector.tensor_scalar_max(pr2, pr, EPS * EPS)
            s = eppool.tile([P, n_tiles], F32)
            nc.scalar.activation(s, pr2, Act.Sqrt, bias=zero_t)
            rs = eppool.tile([P, n_tiles], F32)
            nc.vector.reciprocal(rs, s)
            dd = eppool.tile([P, n_tiles], F32)
            nc.vector.tensor_mul(dd, dot_s, rs)
            pos = eppool.tile([P, n_tiles], F32)  # 1 - d
            nc.scalar.activation(pos, dd, Act.Copy, bias=1.0, scale=-1.0)
            neg = eppool.tile([P, n_tiles], F32)  # relu(d - margin)
            nc.scalar.activation(neg, dd, Act.Relu, bias=negm_t)
            per = eppool.tile([P, n_tiles], F32)
            nc.vector.select(per, mask_t, pos, neg)
            row = eppool.tile([P, 1], F32)
            nc.vector.reduce_sum(row, per, axis=mybir.AxisListType.X)
            ps = psumpool.tile([1, 1], F32)
            nc.tensor.matmul(out=ps, lhsT=row, rhs=ones_t, start=True, stop=True)
            res = eppool.tile([1, 1], F32)
            nc.scalar.copy(res, ps)
            nc.sync.dma_start(out=o_dram.ap(), in_=res)

    nc.compile()
    return nc


_cached_nc = None


def _get_nc():
    global _cached_nc
    if _cached_nc is None:
        _cached_nc = build()
    return _cached_nc


def _make_in_maps(x, y, p, rows_per_core=ROWS_PER_CORE):
    x = np.ascontiguousarray(np.asarray(x, dtype=np.float32))
    y = np.ascontiguousarray(np.asarray(y, dtype=np.float32))
    m_full = (np.asarray(p) == 1).astype(np.uint8)
    n_tiles = rows_per_core // P
    in_maps = []
    for c in range(N_CORES):
        sl = slice(c * rows_per_core, (c + 1) * rows_per_core)
        m_c = np.ascontiguousarray(m_full[sl].reshape(n_tiles, P).T)
        in_maps.append(
            {
                "x": np.ascontiguousarray(x[sl]),
                "y": np.ascontiguousarray(y[sl]),
                "m": m_c,
            }
        )
    return in_maps


def run(x, y, p, trace=False):
    """Returns (loss_scalar_f32, exec_time_ns_or_None)."""
    nc = _get_nc()
    in_maps = _make_in_maps(x, y, p)
    res = run_bass_kernel_spmd(nc, in_maps, list(range(N_CORES)), trace=trace)
    partials = np.array(
        [r["out"][0, 0] for r in res.results], dtype=np.float32
    )
    total = np.float32(np.sum(partials, dtype=np.float32))
    return total, res.exec_time_ns


def kernel(x, y, p):
    total, _ = run(x, y, p)
    return total
